# revision 1
# baseline (speedup 1.0000x reference)
"""Trainium2 Bass kernel for nn_HardSigmoidRT.

Computes out = where(z < e2, e0, where(z <= e3, e0 + (e1-e0)/(e3-e2)*(z-e2), e1))
where eta=[e0,e1,e2,e3] comes from a tiny per-sample MLP on [N,4] inputs.

Strategy:
  - The eta MLP is O(N*4*64) flops -> computed on host in float64 numpy.
  - The piecewise-linear map over z [128,1024,512] f32 (256 MiB in/out) is the
    real work: pure data parallelism over the sample axis N across 8 cores.
  - Per core: 16 samples, each sample = 1024*512 = 128*4096 f32, processed as
    one [128, 4096] SBUF tile (2 MiB DMAs).
  - Device math per tile (per-partition scalar operands from a small params
    tile):  t = (z - e2) * slope   (DVE tensor_scalar, 2 ops)
            u = min(max(t, 0), e1-e0)  (DVE tensor_scalar, 2 ops, in-place)
            out = u + e0           (ACT Identity with per-partition bias)
    This matches the reference's float32 op ordering exactly in the left
    plateau and interior; right-plateau deviations are <= ~2 ulp of e1.
"""

import numpy as np

N = 128
H, W = 1024, 512
NCORES = 8
NPER = N // NCORES            # 16 samples per core
P = 128                       # SBUF partitions
SAMPLE = H * W                # 524288 = P * 4096
FREE = SAMPLE // P            # 4096
ROWS = NPER * P               # 2048 rows per core

_cache = {}


def _eta_host(rt_, noise, X_min, X_max, Y_min, Y_max, W1, b1, W2, b2):
    """float64 mirror of the reference _eta; returns float32 [N,4]."""
    rt = rt_.astype(np.float64)
    sig = 1.0 / (1.0 + np.exp(-rt))
    RTn = np.concatenate([sig, np.zeros(1)])
    Xmin = X_min.astype(np.float64)
    Xmax = X_max.astype(np.float64)
    RT = RTn * (Xmax - Xmin) + Xmin
    RT_noisy = RT[None, :] * noise.astype(np.float64)
    ext = np.stack(
        [RT_noisy[:, 0], RT_noisy[:, 1], RT_noisy[:, 2],
         RT_noisy[:, 1] / RT_noisy[:, 2]], axis=1)
    xn = (ext - Xmin) / (Xmax - Xmin)
    h = np.maximum(xn @ W1.astype(np.float64) + b1.astype(np.float64), 0.0)
    logits = h @ W2.astype(np.float64) + b2.astype(np.float64)
    eta_n = 1.0 / (1.0 + np.exp(-logits))
    eta = eta_n * (Y_max.astype(np.float64) - Y_min.astype(np.float64)) \
        + Y_min.astype(np.float64)
    return eta.astype(np.float32)


def _build_module(reps=1, tile_free=FREE, zbufs=3, obufs=3, use_act=True,
                  store_engine="scalar", rep_barrier=False, mode="full"):
    """Build the SPMD Bass module.

    tile_free: free-dim width of each [128, tile_free] tile. Each sample is
      [128, 4096] in DRAM; tile_free can divide 4096 (several tiles/sample)
      or be a multiple of 4096 (several samples/tile: sample boundaries then
      fall on partition boundaries, params laid out per-partition).
    """
    import concourse.bacc as bacc
    import concourse.mybir as mybir
    from concourse.tile import TileContext

    f32 = mybir.dt.float32
    Alu = mybir.AluOpType
    Act = mybir.ActivationFunctionType

    nc = bacc.Bacc(trn_type="TRN2", target_bir_lowering=False, debug=False,
                   num_devices=NCORES)
    total = ROWS * FREE                      # elements per core
    assert total % (P * tile_free) == 0
    assert SAMPLE % tile_free == 0           # partition spans stay in-sample
    ntiles = total // (P * tile_free)        # tiles per core
    # rows of the [ROWS, FREE] DRAM view covered by one tile-partition-row
    z_in = nc.dram_tensor("z", [ntiles * P, tile_free], f32,
                          kind="ExternalInput")
    # params: per (tile, partition) quad -> [P, 4*ntiles] with column block
    # 4t..4t+4 holding (e2, slope, d, e0) for tile t, partition p.
    par_in = nc.dram_tensor("params", [P, 4 * ntiles], f32,
                            kind="ExternalInput")
    out = nc.dram_tensor("out", [ntiles * P, tile_free], f32,
                         kind="ExternalOutput")
    st = getattr(nc, store_engine)

    inplace = (obufs == 0)
    with TileContext(nc) as tc:
        with tc.tile_pool(name="const", bufs=1) as cpool, \
             tc.tile_pool(name="zp", bufs=zbufs) as zpool, \
             tc.tile_pool(name="op", bufs=max(obufs, 1)) as opool:
            par = cpool.tile([P, 4 * ntiles], f32)
            nc.sync.dma_start(out=par[:], in_=par_in[:])
            for r in range(reps):
                if rep_barrier and r > 0:
                    tc.strict_bb_all_engine_barrier()
                for t in range(ntiles):
                    e2 = par[:, 4 * t + 0:4 * t + 1]
                    sl = par[:, 4 * t + 1:4 * t + 2]
                    d = par[:, 4 * t + 2:4 * t + 3]
                    e0 = par[:, 4 * t + 3:4 * t + 4]
                    zt = zpool.tile([P, tile_free], f32, tag="zt")
                    nc.sync.dma_start(out=zt[:], in_=z_in[t * P:(t + 1) * P, :])
                    if mode == "load":
                        continue
                    if mode == "copy":
                        st.dma_start(out=out[t * P:(t + 1) * P, :], in_=zt[:])
                        continue
                    # t1 = (z - e2) * slope
                    nc.vector.tensor_scalar(zt[:], zt[:], e2, sl,
                                            Alu.subtract, Alu.mult)
                    # u = min(max(t1, 0), e1-e0)
                    nc.vector.tensor_scalar(zt[:], zt[:], 0.0, d,
                                            Alu.max, Alu.min)
                    ot = zt if inplace else opool.tile([P, tile_free], f32,
                                                       tag="ot")
                    # out = u + e0
                    if use_act:
                        nc.scalar.activation(ot[:], zt[:], Act.Identity,
                                             bias=e0, scale=1.0)
                    else:
                        nc.vector.tensor_scalar(ot[:], zt[:], e0, None,
                                                Alu.add)
                    st.dma_start(out=out[t * P:(t + 1) * P, :], in_=ot[:])
    nc.compile()
    return nc


# chosen kernel configuration (shared by kernel() and bench harnesses)
KCONF = dict(tile_free=FREE, zbufs=4, obufs=3, use_act=True,
             store_engine="scalar")


def _get_module():
    if "nc" not in _cache:
        _cache["nc"] = _build_module(**KCONF)
    return _cache["nc"]


def make_in_maps(z, quad, tile_free):
    """Shard z + per-sample quads into per-core input maps for the module
    built with the given tile_free. quad: [N, 4] f32 (e2, slope, d, e0)."""
    total = ROWS * FREE
    ntiles = total // (P * tile_free)
    # sample index owning each (tile, partition) row of a core's z view
    rows = np.arange(ntiles * P)
    sample_of_row = (rows * tile_free) // SAMPLE          # [ntiles*P]
    sample_of_row = sample_of_row.reshape(ntiles, P)      # [ntiles, P]
    in_maps = []
    for c in range(NCORES):
        zc = z[c * NPER:(c + 1) * NPER].reshape(ntiles * P, tile_free)
        qc = quad[c * NPER:(c + 1) * NPER]                # [NPER, 4]
        pc = qc[sample_of_row]                            # [ntiles, P, 4]
        pc = np.ascontiguousarray(
            pc.transpose(1, 0, 2).reshape(P, 4 * ntiles), dtype=np.float32)
        in_maps.append({"z": zc, "params": pc})
    return in_maps


def make_quad(inputs):
    eta = _eta_host(inputs["rt_"], inputs["noise"], inputs["X_min"],
                    inputs["X_max"], inputs["Y_min"], inputs["Y_max"],
                    inputs["W1"], inputs["b1"], inputs["W2"], inputs["b2"])
    e0, e1, e2, e3 = eta[:, 0], eta[:, 1], eta[:, 2], eta[:, 3]
    # f32 ops, same order as reference: slope = (e1-e0)/(e3-e2)
    d = e1 - e0
    slope = d / (e3 - e2)
    return np.stack([e2, slope, d, e0], axis=1)           # [N, 4] f32


def kernel(**inputs):
    from concourse.bass_utils import run_bass_kernel_spmd

    # jax arrays (x64-disabled) would silently downcast in _eta_host;
    # normalize everything to real numpy first.
    inputs = {k: np.asarray(v) for k, v in inputs.items()}
    z = np.ascontiguousarray(inputs["z"], dtype=np.float32)
    quad = make_quad(inputs)
    nc = _get_module()
    in_maps = make_in_maps(z, quad, KCONF["tile_free"])
    res = run_bass_kernel_spmd(nc, in_maps, core_ids=list(range(NCORES)))
    outs = [r["out"].reshape(NPER, H, W) for r in res.results]
    return np.concatenate(outs, axis=0)



# revision 5
# speedup vs baseline: 2.4512x; 2.4512x over previous
"""Trainium2 Bass kernel for nn_HardSigmoidRT.

Computes out = where(z < e2, e0, where(z <= e3, e0 + (e1-e0)/(e3-e2)*(z-e2), e1))
where eta=[e0,e1,e2,e3] comes from a tiny per-sample MLP on [N,4] inputs.

Strategy:
  - The eta MLP is O(N*4*64) flops -> computed on host in float64 numpy.
  - The piecewise-linear map over z [128,1024,512] f32 (256 MiB in/out) is the
    real work: pure data parallelism over the sample axis N across 8 cores.
  - Per core: 16 samples, each sample = 1024*512 = 128*4096 f32 viewed as
    [128, 4096]; HBM traffic is 64 MiB/core, the per-NC HBM roofline
    (~358 GB/s) makes this memory-bound at ~187 us.
  - Device math ("2op", both on DVE, in-place):
        t   = z * s + c          with s = (e1-e0)/(e3-e2), c = e0 - s*e2
        out = min(max(t, e0), e0 + (e1-e0))
    i.e. the affine map pre-folded so the whole piecewise-linear function is
    two tensor_scalar instructions; ACT and SP stay free to issue store/load
    DMAs without compute in their queues.
  - Tile schedule: per-sample [128, w] column chunks; the first/last samples
    can be split into narrower chunks (head/tail taper) so the pipeline fills
    quickly and the final store chain after the last compute is short.
"""

import numpy as np

N = 128
H, W = 1024, 512
NCORES = 8
NPER = N // NCORES            # 16 samples per core
P = 128                       # SBUF partitions
SAMPLE = H * W                # 524288 = P * 4096
FREE = SAMPLE // P            # 4096
ROWS = NPER * P               # 2048 rows per core

_cache = {}


def _eta_host(rt_, noise, X_min, X_max, Y_min, Y_max, W1, b1, W2, b2):
    """float64 mirror of the reference _eta; returns float32 [N,4]."""
    rt = rt_.astype(np.float64)
    sig = 1.0 / (1.0 + np.exp(-rt))
    RTn = np.concatenate([sig, np.zeros(1)])
    Xmin = X_min.astype(np.float64)
    Xmax = X_max.astype(np.float64)
    RT = RTn * (Xmax - Xmin) + Xmin
    RT_noisy = RT[None, :] * noise.astype(np.float64)
    ext = np.stack(
        [RT_noisy[:, 0], RT_noisy[:, 1], RT_noisy[:, 2],
         RT_noisy[:, 1] / RT_noisy[:, 2]], axis=1)
    xn = (ext - Xmin) / (Xmax - Xmin)
    h = np.maximum(xn @ W1.astype(np.float64) + b1.astype(np.float64), 0.0)
    logits = h @ W2.astype(np.float64) + b2.astype(np.float64)
    eta_n = 1.0 / (1.0 + np.exp(-logits))
    eta = eta_n * (Y_max.astype(np.float64) - Y_min.astype(np.float64)) \
        + Y_min.astype(np.float64)
    return eta.astype(np.float32)


def make_quad(inputs):
    """[N, 4] f32 eta = (e0, e1, e2, e3) per sample."""
    return _eta_host(inputs["rt_"], inputs["noise"], inputs["X_min"],
                     inputs["X_max"], inputs["Y_min"], inputs["Y_max"],
                     inputs["W1"], inputs["b1"], inputs["W2"], inputs["b2"])


def _params_from_eta(eta, math):
    """Per-sample param quad [N, 4] f32 for the chosen device math."""
    e0 = eta[:, 0].astype(np.float64)
    e1 = eta[:, 1].astype(np.float64)
    e2 = eta[:, 2].astype(np.float64)
    e3 = eta[:, 3].astype(np.float64)
    # match the reference's f32 op order for the slope
    d32 = (eta[:, 1] - eta[:, 0]).astype(np.float32)
    s32 = (d32 / (eta[:, 3] - eta[:, 2]).astype(np.float32)).astype(np.float32)
    s = s32.astype(np.float64)
    if math == "3op":
        q = np.stack([e2, s, d32.astype(np.float64), e0], axis=1)
    elif math == "2op":
        c = e0 - s * e2
        q = np.stack([s, c, e0, e0 + d32.astype(np.float64)], axis=1)
    elif math == "split":
        cme0 = -s * e2
        q = np.stack([s, cme0, d32.astype(np.float64), e0], axis=1)
    elif math in ("copy", "load"):
        q = np.stack([e0, e1, e2, e3], axis=1)
    else:
        raise ValueError(math)
    return q.astype(np.float32)


def _schedule(tile_free, head, tail):
    """List of (sample, col0, width) chunks over the [NPER*P, FREE] view."""
    sched = []
    for smp in range(NPER):
        if smp == 0 and head:
            w = head
        elif smp == NPER - 1 and tail:
            w = tail
        else:
            w = tile_free
        assert FREE % w == 0
        for c0 in range(0, FREE, w):
            sched.append((smp, c0, w))
    return sched


def _build_module(reps=1, math="2op", tile_free=FREE, zbufs=10, obufs=0,
                  head=None, tail=None, store_engine="scalar", mode="full",
                  mixq=False):
    """Build the SPMD Bass module.

    math: "3op" (baseline DVE+ACT), "2op" (two DVE tensor_scalar, in-place),
      "split" (ACT relu-affine + DVE min/add), or mode="copy"/"load" ceilings.
    tile_free <= FREE: per-sample column chunks (with optional head/tail
      taper widths for the first/last sample). tile_free > FREE: multi-sample
      row-block tiles (params per partition).
    """
    import concourse.bacc as bacc
    import concourse.mybir as mybir
    from concourse.tile import TileContext

    f32 = mybir.dt.float32
    Alu = mybir.AluOpType
    Act = mybir.ActivationFunctionType

    nc = bacc.Bacc(trn_type="TRN2", target_bir_lowering=False, debug=False,
                   num_devices=NCORES)
    if mode in ("copy", "load"):
        math = mode

    rowblock = tile_free > FREE
    if rowblock:
        total = ROWS * FREE
        assert total % (P * tile_free) == 0
        ntiles = total // (P * tile_free)
        z_in = nc.dram_tensor("z", [ntiles * P, tile_free], f32,
                              kind="ExternalInput")
        par_in = nc.dram_tensor("params", [P, 4 * ntiles], f32,
                                kind="ExternalInput")
        out = nc.dram_tensor("out", [ntiles * P, tile_free], f32,
                             kind="ExternalOutput")
        npar = ntiles
        sched = [(t, 0, tile_free) for t in range(ntiles)]
        max_w = tile_free
    else:
        z_in = nc.dram_tensor("z", [ROWS, FREE], f32, kind="ExternalInput")
        par_in = nc.dram_tensor("params", [P, 4 * NPER], f32,
                                kind="ExternalInput")
        out = nc.dram_tensor("out", [ROWS, FREE], f32, kind="ExternalOutput")
        npar = NPER
        sched = _schedule(tile_free, head, tail)
        max_w = max(w for _, _, w in sched)

    st = getattr(nc, store_engine)

    with TileContext(nc) as tc:
        with tc.tile_pool(name="const", bufs=1) as cpool, \
             tc.tile_pool(name="zp", bufs=zbufs) as zpool, \
             tc.tile_pool(name="op", bufs=max(obufs, 1)) as opool:
            par = cpool.tile([P, 4 * npar], f32)
            nc.sync.dma_start(out=par[:], in_=par_in[:])
            for _ in range(reps):
                for i, (t, c0, w) in enumerate(sched):
                    if mixq:
                        ld = nc.sync if i % 2 == 0 else nc.scalar
                        st = nc.scalar if i % 2 == 0 else nc.sync
                    else:
                        ld = nc.sync
                    if rowblock:
                        src = z_in[t * P:(t + 1) * P, :]
                        dst = out[t * P:(t + 1) * P, :]
                    else:
                        src = z_in[t * P:(t + 1) * P, c0:c0 + w]
                        dst = out[t * P:(t + 1) * P, c0:c0 + w]
                    p0 = par[:, 4 * t + 0:4 * t + 1]
                    p1 = par[:, 4 * t + 1:4 * t + 2]
                    p2 = par[:, 4 * t + 2:4 * t + 3]
                    p3 = par[:, 4 * t + 3:4 * t + 4]
                    zt = zpool.tile([P, max_w], f32, tag="zt")
                    zv = zt[:, :w]
                    ld.dma_start(out=zv, in_=src)
                    if math == "load":
                        continue
                    if math == "copy":
                        st.dma_start(out=dst, in_=zv)
                        continue
                    if math == "2op":
                        # t = z*s + c ; out = min(max(t, e0), e1')
                        nc.vector.tensor_scalar(zv, zv, p0, p1,
                                                Alu.mult, Alu.add)
                        nc.vector.tensor_scalar(zv, zv, p2, p3,
                                                Alu.max, Alu.min)
                        ov = zv
                    elif math == "3op":
                        # (s=p1) t1 = (z-e2)*s ; u = min(max(t1,0), d)
                        nc.vector.tensor_scalar(zv, zv, p0, p1,
                                                Alu.subtract, Alu.mult)
                        nc.vector.tensor_scalar(zv, zv, 0.0, p2,
                                                Alu.max, Alu.min)
                        if obufs > 0:
                            ot = opool.tile([P, max_w], f32, tag="ot")
                            ov = ot[:, :w]
                        else:
                            ov = zv
                        nc.scalar.activation(ov, zv, Act.Identity,
                                             bias=p3, scale=1.0)
                    elif math == "split":
                        # u = relu(z*s + (c-e0)) on ACT; out = min(u,d)+e0
                        if obufs > 0:
                            ot = opool.tile([P, max_w], f32, tag="ot")
                            ov = ot[:, :w]
                        else:
                            ov = zv
                        nc.scalar.activation(ov, zv, Act.Relu,
                                             bias=p1, scale=p0)
                        nc.vector.tensor_scalar(ov, ov, p2, p3,
                                                Alu.min, Alu.add)
                    else:
                        raise ValueError(math)
                    st.dma_start(out=dst, in_=ov)
    nc.compile()
    return nc


# chosen kernel configuration (shared by kernel() and bench harnesses)
KCONF = dict(math="2op", tile_free=FREE, zbufs=10, obufs=0,
             head=None, tail=None, store_engine="scalar")


def _get_module():
    if "nc" not in _cache:
        _cache["nc"] = _build_module(**KCONF)
    return _cache["nc"]


def make_in_maps(z, eta, kconf):
    """Shard z + per-sample eta into per-core input maps for the module
    built with the given kconf. eta: [N, 4] f32 (e0, e1, e2, e3)."""
    quad = _params_from_eta(eta, kconf["math"])
    tile_free = kconf["tile_free"]
    in_maps = []
    if tile_free > FREE:
        total = ROWS * FREE
        ntiles = total // (P * tile_free)
        rows = np.arange(ntiles * P)
        sample_of_row = (rows * tile_free) // SAMPLE
        sample_of_row = sample_of_row.reshape(ntiles, P)
        for c in range(NCORES):
            zc = z[c * NPER:(c + 1) * NPER].reshape(ntiles * P, tile_free)
            qc = quad[c * NPER:(c + 1) * NPER]
            pc = qc[sample_of_row]                        # [ntiles, P, 4]
            pc = np.ascontiguousarray(
                pc.transpose(1, 0, 2).reshape(P, 4 * ntiles), dtype=np.float32)
            in_maps.append({"z": zc, "params": pc})
    else:
        for c in range(NCORES):
            zc = z[c * NPER:(c + 1) * NPER].reshape(ROWS, FREE)
            qc = quad[c * NPER:(c + 1) * NPER]            # [NPER, 4]
            pc = np.ascontiguousarray(
                np.broadcast_to(qc.reshape(1, 4 * NPER), (P, 4 * NPER)),
                dtype=np.float32)
            in_maps.append({"z": zc, "params": pc})
    return in_maps


def kernel(**inputs):
    from concourse.bass_utils import run_bass_kernel_spmd

    # jax arrays (x64-disabled) would silently downcast in _eta_host;
    # normalize everything to real numpy first.
    inputs = {k: np.asarray(v) for k, v in inputs.items()}
    z = np.ascontiguousarray(inputs["z"], dtype=np.float32)
    eta = make_quad(inputs)
    nc = _get_module()
    in_maps = make_in_maps(z, eta, KCONF)
    res = run_bass_kernel_spmd(nc, in_maps, core_ids=list(range(NCORES)))
    outs = [r["out"].reshape(NPER, H, W) for r in res.results]
    return np.concatenate(outs, axis=0)
